# revision 5
# baseline (speedup 1.0000x reference)
"""Bahdanau additive attention on 8 TRN2 NeuronCores (data-parallel over batch).

reference math (per batch b):
    Ws = enc @ W_a                      [Te, H]
    Uh = dec @ U_a                      [Td, H]
    e[d, t]  = sum_h V[h] * tanh(Ws[t, h] + Uh[d, h])
    e = softmax(e, axis=t)              [Td, Te]
    c = e @ enc                         [Td, H]
returns (c, e).

Per-core layout strategy (2 batches/core):
  - everything bf16 on the compute path, fp32 accumulation in PSUM
  - WsT [h, t] and UhT [h, d] (h on partitions) so the broadcast-add
    S_d = WsT + UhT[:, d] is a DVE tensor_scalar (per-partition operand, 4x mode)
  - tanh on ScalarE over big batched tiles (the bottleneck: 33.5M elems/core)
  - V-reduction over h via TensorE: stationary = tanh tile [h=128, t=128],
    moving = V chunk [128, 1] -> psum column e_T[t, d]
  - softmax denominators via ones-matmul on the e_T stationary; normalization
    folded into the context/e outputs as a per-partition reciprocal multiply
"""

from contextlib import ExitStack

import numpy as np

import concourse.bass as bass
import concourse.tile as tile
from concourse import bacc, mybir
from concourse.bass import ts
from concourse.bass_utils import run_bass_kernel_spmd
from concourse.masks import make_identity

B, TE, TD, H = 16, 256, 128, 512
NCORES = 8
BL = B // NCORES  # local batches per core
HC = H // 128     # h chunks
KC = H // 128     # contraction chunks
TC = TE // 128    # encoder-step chunks
DBLK = 4          # (b, d) pairs per tanh batch

F32 = mybir.dt.float32
BF16 = mybir.dt.bfloat16


def _attention(tc, enc, dec, w, u, v, out_c, out_e):
    nc = tc.nc
    TANH = mybir.ActivationFunctionType.Tanh
    EXP = mybir.ActivationFunctionType.Exp

    with ExitStack() as ctx:
        consts = ctx.enter_context(tc.tile_pool(name="consts", bufs=1))
        stage = ctx.enter_context(tc.tile_pool(name="stage", bufs=2))
        persist = ctx.enter_context(tc.tile_pool(name="persist", bufs=1))
        spool = ctx.enter_context(tc.tile_pool(name="spool", bufs=3))
        outp = ctx.enter_context(tc.tile_pool(name="outp", bufs=2))
        psum_tp = ctx.enter_context(tc.tile_pool(name="psum_tp", bufs=2, space="PSUM"))
        psum_mm = ctx.enter_context(tc.tile_pool(name="psum_mm", bufs=2, space="PSUM"))
        psum_et = ctx.enter_context(tc.tile_pool(name="psum_et", bufs=1, space="PSUM"))
        psum_sm = ctx.enter_context(tc.tile_pool(name="psum_sm", bufs=1, space="PSUM"))

        ident = consts.tile([128, 128], BF16, tag="ident", name="ident")
        make_identity(nc, ident)
        ones = consts.tile([128, 1], BF16, tag="ones", name="ones")
        nc.vector.memset(ones, 1.0)
        zbias = consts.tile([128, 1], F32, tag="zbias", name="zbias")
        nc.vector.memset(zbias, 0.0)

        # ---- load inputs, cast to bf16 ----
        def load_cast(dram_ap, shape, tag):
            f32t = stage.tile(shape, F32, tag="stg")
            nc.sync.dma_start(out=f32t, in_=dram_ap)
            bft = persist.tile(shape, BF16, tag=tag)
            nc.vector.tensor_copy(out=bft, in_=f32t)
            return bft

        enc_bf = [[load_cast(enc[b, ts(t, 128), :], [128, H], f"encbf_{b}_{t}")
                   for t in range(TC)] for b in range(BL)]
        dec_bf = [load_cast(dec[b], [128, H], f"decbf_{b}") for b in range(BL)]
        w_bf = [load_cast(w[ts(k, 128), :], [128, H], f"wbf_{k}") for k in range(KC)]
        u_bf = [load_cast(u[ts(k, 128), :], [128, H], f"ubf_{k}") for k in range(KC)]
        v_bf = load_cast(v.rearrange("(c p) o -> p (c o)", p=128), [128, HC], "vbf")

        # ---- transposes: encT[k, t], decT[k, d] ----
        encT = [[persist.tile([128, TE], BF16, tag=f"encT_{b}_{k}", name=f"encT_{b}_{k}")
                 for k in range(KC)] for b in range(BL)]
        decT = [[persist.tile([128, TD], BF16, tag=f"decT_{b}_{k}", name=f"decT_{b}_{k}")
                 for k in range(KC)] for b in range(BL)]
        for b in range(BL):
            for k in range(KC):
                for t in range(TC):
                    ps = psum_tp.tile([128, 128], BF16, tag="tp", name="tp")
                    nc.tensor.transpose(ps, enc_bf[b][t][:, ts(k, 128)], ident)
                    nc.vector.tensor_copy(out=encT[b][k][:, ts(t, 128)], in_=ps)
                ps = psum_tp.tile([128, 128], BF16, tag="tp", name="tp")
                nc.tensor.transpose(ps, dec_bf[b][:, ts(k, 128)], ident)
                nc.vector.tensor_copy(out=decT[b][k], in_=ps)

        # ---- WsT[b][m] = (enc @ W)^T  [h=128, TE];  UhT[b][m]  [h=128, TD] ----
        wst = [[persist.tile([128, TE], BF16, tag=f"wst_{b}_{m}", name=f"wst_{b}_{m}") for m in range(HC)]
               for b in range(BL)]
        uht = [[persist.tile([128, TD], F32, tag=f"uht_{b}_{m}", name=f"uht_{b}_{m}") for m in range(HC)]
               for b in range(BL)]
        for b in range(BL):
            for m in range(HC):
                pws = psum_mm.tile([128, TE], F32, tag="pmm", name="pmm")
                for k in range(KC):
                    nc.tensor.matmul(pws, w_bf[k][:, ts(m, 128)], encT[b][k],
                                     start=(k == 0), stop=(k == KC - 1))
                nc.vector.tensor_copy(out=wst[b][m], in_=pws)
                puh = psum_mm.tile([128, TD], F32, tag="pmm", name="pmm")
                for k in range(KC):
                    nc.tensor.matmul(puh, u_bf[k][:, ts(m, 128)], decT[b][k],
                                     start=(k == 0), stop=(k == KC - 1))
                nc.vector.tensor_copy(out=uht[b][m], in_=puh)

        # ---- main loop: S = WsT + UhT[:, d]; tanh; V-reduce -> e_T columns ----
        # psum_eT[b]: [t(128), tc, d] accumulated one column per (b, d)
        psum_eT = [psum_et.tile([128, TC, TD], F32, tag=f"eT_{b}", name=f"eT_{b}") for b in range(BL)]
        PAIRS = BL * TD
        for blk in range(PAIRS // DBLK):
            s_tile = spool.tile([128, DBLK * HC * TE], BF16, tag="s", name="s")
            for j in range(DBLK):
                pair = blk * DBLK + j
                b, d = pair % BL, pair // BL
                for m in range(HC):
                    nc.vector.tensor_scalar_add(
                        s_tile[:, ts(j * HC + m, TE)],
                        wst[b][m],
                        uht[b][m][:, d:d + 1],
                    )
            nc.scalar.activation(out=s_tile, in_=s_tile, func=TANH, bias=zbias)
            for j in range(DBLK):
                pair = blk * DBLK + j
                b, d = pair % BL, pair // BL
                for t in range(TC):
                    for m in range(HC):
                        nc.tensor.matmul(
                            psum_eT[b][:, t, d:d + 1],
                            s_tile[:, (j * HC + m) * TE + t * 128:
                                      (j * HC + m) * TE + (t + 1) * 128],
                            v_bf[:, m:m + 1],
                            start=(m == 0), stop=(m == HC - 1),
                        )

        # ---- softmax + context ----
        psum_s = psum_sm.tile([128, BL], F32, tag="sums", name="sums")
        for b in range(BL):
            expT = outp.tile([128, TC, 128], BF16, tag=f"expT_{b}", name=f"expT_{b}")
            nc.scalar.activation(out=expT, in_=psum_eT[b], func=EXP, bias=zbias)
            psum_c = psum_mm.tile([128, H], F32, tag="pmm", name="pmm")
            for t in range(TC):
                nc.tensor.matmul(psum_s[:, b:b + 1], expT[:, t, :], ones,
                                 start=(t == 0), stop=(t == TC - 1))
                nc.tensor.matmul(psum_c, expT[:, t, :], enc_bf[b][t],
                                 start=(t == 0), stop=(t == TC - 1))
            recip = outp.tile([128, 1], F32, tag=f"recip_{b}", name=f"recip_{b}")
            nc.vector.reciprocal(recip, psum_s[:, b:b + 1])
            c_sb = outp.tile([128, H], F32, tag="c_sb", name="c_sb")
            nc.vector.tensor_scalar_mul(c_sb, psum_c, recip)
            nc.sync.dma_start(out=out_c[b], in_=c_sb)
            e_sb = outp.tile([128, TE], F32, tag="e_sb", name="e_sb")
            for t in range(TC):
                pse = psum_tp.tile([128, 128], BF16, tag="tp", name="tp")
                nc.tensor.transpose(pse, expT[:, t, :], ident)
                nc.vector.tensor_scalar_mul(e_sb[:, ts(t, 128)], pse, recip)
            nc.sync.dma_start(out=out_e[b], in_=e_sb)


def build_bass():
    nc = bacc.Bacc("TRN2", target_bir_lowering=False, debug=False)
    enc = nc.dram_tensor("enc", [BL, TE, H], F32, kind="ExternalInput").ap()
    dec = nc.dram_tensor("dec", [BL, TD, H], F32, kind="ExternalInput").ap()
    w = nc.dram_tensor("w", [H, H], F32, kind="ExternalInput").ap()
    u = nc.dram_tensor("u", [H, H], F32, kind="ExternalInput").ap()
    v = nc.dram_tensor("v", [H, 1], F32, kind="ExternalInput").ap()
    out_c = nc.dram_tensor("out_c", [BL, TD, H], F32, kind="ExternalOutput").ap()
    out_e = nc.dram_tensor("out_e", [BL, TD, TE], F32, kind="ExternalOutput").ap()
    with tile.TileContext(nc) as tc:
        _attention(tc, enc, dec, w, u, v, out_c, out_e)
    nc.compile()
    return nc


_NC_CACHE = None


def _get_nc():
    global _NC_CACHE
    if _NC_CACHE is None:
        _NC_CACHE = build_bass()
    return _NC_CACHE


def make_in_maps(encoder_out_seq, decoder_out_seq, W_a, U_a, V_a):
    enc = np.ascontiguousarray(np.asarray(encoder_out_seq, dtype=np.float32))
    dec = np.ascontiguousarray(np.asarray(decoder_out_seq, dtype=np.float32))
    w = np.ascontiguousarray(np.asarray(W_a, dtype=np.float32))
    u = np.ascontiguousarray(np.asarray(U_a, dtype=np.float32))
    v = np.ascontiguousarray(np.asarray(V_a, dtype=np.float32))
    return [
        {"enc": enc[i * BL:(i + 1) * BL], "dec": dec[i * BL:(i + 1) * BL],
         "w": w, "u": u, "v": v}
        for i in range(NCORES)
    ]


def run(in_maps, **kwargs):
    nc = _get_nc()
    res = run_bass_kernel_spmd(nc, in_maps, core_ids=list(range(NCORES)), **kwargs)
    c = np.concatenate([r["out_c"] for r in res.results], axis=0)
    e = np.concatenate([r["out_e"] for r in res.results], axis=0)
    return (c, e), res


def kernel(encoder_out_seq, decoder_out_seq, W_a, U_a, V_a):
    out, _ = run(make_in_maps(encoder_out_seq, decoder_out_seq, W_a, U_a, V_a))
    return out
